# revision 4
# baseline (speedup 1.0000x reference)
"""Edge-parallel GNN message passing on 8 Trainium2 NeuronCores.

Strategy (host-permuted, fully core-independent):
  * Sort edges by destination node. Pack whole destination segments into
    128-edge tiles (padding tiles so no segment ever spans a tile). Each
    tile therefore owns a disjoint set of destination nodes; tiles are
    dealt contiguously to the 8 cores -> no collective needed.
  * Per tile, on device:
      stage 1: 32 matmuls, each computing 4 edges' (x_src @ A_e) via a
               block-diagonal x operand (K=128 = 4 edges x 32 dims):
               out[32f, 4e] = A_block[128,32].T-contraction with x_block.
      transpose [32,128] -> [128,32] on the PE (identity matmul).
      stage 2: segment-sum via a one-hot selector matmul S.T @ msg, where
               S[e, m] = (rank[e] == m) is built on-device with a DVE
               is_equal against an iota tile. Slot ranks and 1/count are
               precomputed on host and streamed in (tiny).
      epilogue: mean = sum * recip (ACT, per-partition scale), + bias
               (GPSIMD), relu (ACT), DMA out.
  * Host scatters the per-(tile,slot) rows to their node ids; isolated
    nodes get relu(bias).

The 2 GB a_in stream dominates: ~256 MB/core, fp32, fully sequential.
"""

import math
import os
from contextlib import ExitStack

import numpy as np

import concourse.bass as bass  # noqa: F401  (import registers bass types)
import concourse.tile as tile
from concourse import bacc, mybir
from concourse.bass_utils import run_bass_kernel_spmd

F32 = mybir.dt.float32
NCORES = 8
D = 32
EPT = 128          # edges per tile
GPT = EPT // 4     # stage-1 matmul groups per tile


def _pack_segments(counts):
    """Greedy-pack whole segments (counts <= EPT each) into EPT-slot tiles.
    Returns (tile_id, slot) per segment and the number of tiles."""
    n = len(counts)
    tile_id = np.empty(n, np.int64)
    slot = np.empty(n, np.int64)
    t = 0
    used = 0
    nseg = 0
    for i in range(n):
        c = counts[i]
        if used + c > EPT:
            t += 1
            used = 0
            nseg = 0
        tile_id[i] = t
        slot[i] = nseg
        used += c
        nseg += 1
    return tile_id, slot, (t + 1 if n else 0)


def _prep(node_states, edge_index, a_in, bias):
    ns = np.asarray(node_states, dtype=np.float32)
    ei = np.asarray(edge_index)
    a = np.asarray(a_in, dtype=np.float32)
    b = np.asarray(bias, dtype=np.float32)
    n_nodes, d = ns.shape
    assert d == D
    E = ei.shape[0]
    src = np.ascontiguousarray(ei[:, 0]).astype(np.int64)
    dst = np.ascontiguousarray(ei[:, 1]).astype(np.int64)

    perm = np.argsort(dst, kind="stable")
    dsts = dst[perm]
    nodes_u, counts = np.unique(dsts, return_counts=True)

    # Oversize segments (in-degree > EPT) fall back to a host computation.
    big = counts > EPT
    host_nodes = nodes_u[big]
    edge_big = np.repeat(big, counts)          # mask over sorted edges
    perm_k = perm[~edge_big]
    nodes_k = nodes_u[~big]
    counts_k = counts[~big]

    tile_id, slot, n_tiles = _pack_segments(counts_k)
    n_tiles = max(n_tiles, 1)
    T = int(math.ceil(n_tiles / NCORES))
    Ttot = T * NCORES

    ek = len(perm_k)
    if ek:
        e_tile = np.repeat(tile_id, counts_k)
        cum_excl = np.concatenate(([0], np.cumsum(counts_k)))[:-1]
        tile_first_seg = np.searchsorted(tile_id, np.arange(n_tiles))
        tile_edge_start = cum_excl[tile_first_seg]
        e_pos = np.arange(ek) - tile_edge_start[e_tile]
        flat = e_tile * EPT + e_pos
    else:
        flat = np.zeros(0, np.int64)

    ei_flat = np.zeros(Ttot * EPT, np.int64)   # pad edges read edge data 0
    if ek:
        ei_flat[flat] = perm_k
    rank_flat = np.full(Ttot * EPT, -1e9, np.float32)
    recip_flat = np.ones(Ttot * EPT, np.float32)
    flatslot = tile_id * EPT + slot
    if ek:
        rank_flat[flat] = np.repeat(slot, counts_k).astype(np.float32)
        recip_flat[flatslot] = (1.0 / counts_k).astype(np.float32)

    # Per-core device arrays; build per core to bound transient memory.
    A_host = np.empty((NCORES, T, 128, GPT * D), np.float32)
    X_host = np.empty((NCORES, T, 4, D, GPT), np.float32)
    ei_r = ei_flat.reshape(NCORES, T * EPT)
    xsrc = src[ei_flat].reshape(NCORES, T * EPT)
    for c in range(NCORES):
        ae = a[ei_r[c]]                              # [T*EPT, D, D]
        A_host[c] = (
            ae.reshape(T, GPT, 4, D, D)
            .transpose(0, 2, 3, 1, 4)                # [t, j, d, g, f]
            .reshape(T, 128, GPT * D)
        )
        del ae
        xg = ns[xsrc[c]]                             # [T*EPT, D]
        X_host[c] = xg.reshape(T, GPT, 4, D).transpose(0, 2, 3, 1)  # [t,j,d,g]
        del xg

    RR_host = np.stack(
        [rank_flat.reshape(NCORES, T, EPT), recip_flat.reshape(NCORES, T, EPT)],
        axis=-1,
    )                                                # [NCORES, T, EPT, 2]
    iota_host = np.tile(np.arange(128, dtype=np.float32), (128, 1))
    ident_host = np.eye(32, dtype=np.float32)
    biasbc_host = np.tile(b, (128, 1)).astype(np.float32)

    in_maps = [
        {
            "a": A_host[c],
            "x": X_host[c],
            "rr": RR_host[c],
            "iota": iota_host,
            "ident": ident_host,
            "biasbc": biasbc_host,
        }
        for c in range(NCORES)
    ]

    # Host fallback rows for oversize segments.
    host_rows = None
    if len(host_nodes):
        eb = perm[edge_big]
        msg = np.einsum("ed,edf->ef", ns[src[eb]], a[eb])
        summed = np.zeros((len(host_nodes), D), np.float32)
        hn_index = {n: i for i, n in enumerate(host_nodes)}
        idx = np.fromiter((hn_index[n] for n in dst[eb]), np.int64, len(eb))
        np.add.at(summed, idx, msg)
        cnt = counts[big].astype(np.float32)[:, None]
        host_rows = np.maximum(summed / cnt + b[None, :], 0.0).astype(np.float32)

    meta = dict(
        n_nodes=n_nodes,
        T=T,
        nodes_k=nodes_k,
        flatslot=flatslot,
        host_nodes=host_nodes,
        host_rows=host_rows,
        bias=b,
    )
    return in_maps, meta


def _build(T, enable_asserts=False):
    nc = bacc.Bacc(
        "TRN2",
        target_bir_lowering=False,
        debug=False,
        enable_asserts=enable_asserts,
        num_devices=NCORES,
    )
    a_d = nc.dram_tensor("a", [T, 128, GPT * D], F32, kind="ExternalInput")
    x_d = nc.dram_tensor("x", [T, 4, D, GPT], F32, kind="ExternalInput")
    rr_d = nc.dram_tensor("rr", [T, EPT, 2], F32, kind="ExternalInput")
    iota_d = nc.dram_tensor("iota", [128, 128], F32, kind="ExternalInput")
    id_d = nc.dram_tensor("ident", [32, 32], F32, kind="ExternalInput")
    bb_d = nc.dram_tensor("biasbc", [128, 32], F32, kind="ExternalInput")
    out_d = nc.dram_tensor("out", [T, EPT, D], F32, kind="ExternalOutput")

    with tile.TileContext(nc) as tc, ExitStack() as ctx:
        cpool = ctx.enter_context(tc.tile_pool(name="const", bufs=1))
        apool = ctx.enter_context(tc.tile_pool(name="apool", bufs=4))
        spool = ctx.enter_context(tc.tile_pool(name="spool", bufs=3))
        wpool = ctx.enter_context(tc.tile_pool(name="wpool", bufs=3))
        opool = ctx.enter_context(tc.tile_pool(name="opool", bufs=3))
        ps_a = ctx.enter_context(tc.tile_pool(name="ps_a", bufs=2, space="PSUM"))
        ps_b = ctx.enter_context(tc.tile_pool(name="ps_b", bufs=2, space="PSUM"))
        ps_c = ctx.enter_context(tc.tile_pool(name="ps_c", bufs=2, space="PSUM"))

        iota_t = cpool.tile([128, 128], F32, tag="iota")
        nc.sync.dma_start(iota_t[:], iota_d[:])
        id_t = cpool.tile([32, 32], F32, tag="ident")
        nc.sync.dma_start(id_t[:], id_d[:])
        bb_t = cpool.tile([128, 32], F32, tag="biasbc")
        nc.sync.dma_start(bb_t[:], bb_d[:])

        # Two persistent block-diagonal x operands; the off-diagonal cells
        # are zeroed once and never written again (DMAs only touch the
        # diagonal bands), so reuse keeps them zero.
        xm = []
        for i in range(2):
            t_ = cpool.tile([128, 128], F32, tag=f"xmega{i}")
            nc.vector.memset(t_[:], 0.0)
            xm.append(t_)

        for t in range(T):
            at = apool.tile([128, GPT * D], F32, tag="a")
            nc.sync.dma_start(at[:], a_d[t])

            # x_mega columns are grouped j-major: col = 32*j + g, so each
            # per-j DMA lands in one contiguous 32x32 block (the diagonal
            # block of band j); off-diagonal blocks stay zero forever.
            x_mega = xm[t % 2]
            for j in range(4):
                nc.sync.dma_start(
                    x_mega[32 * j : 32 * j + 32, 32 * j : 32 * j + 32], x_d[t, j]
                )
            xv = x_mega.rearrange("p (j g) -> p j g", j=4)

            rrt = wpool.tile([EPT, 2], F32, tag="rr")
            nc.sync.dma_start(rrt[:], rr_d[t])

            msgT_ps = ps_a.tile([32, 128], F32, tag="msgT")
            for g in range(GPT):
                nc.tensor.matmul(
                    msgT_ps[:, 4 * g : 4 * g + 4],
                    at[:, 32 * g : 32 * g + 32],
                    xv[:, :, g],
                    start=True,
                    stop=True,
                )
            msgT_sb = wpool.tile([32, 128], F32, tag="msgTsb")
            nc.scalar.copy(msgT_sb[:], msgT_ps[:])

            msg_ps = ps_b.tile([128, 32], F32, tag="msg")
            nc.tensor.transpose(msg_ps[:], msgT_sb[:], id_t[:])
            msg_sb = wpool.tile([128, 32], F32, tag="msgsb")
            nc.vector.tensor_copy(msg_sb[:], msg_ps[:])

            s_t = spool.tile([128, 128], F32, tag="S")
            nc.vector.tensor_scalar(
                s_t[:], iota_t[:], rrt[:, 0:1], None, mybir.AluOpType.is_equal
            )

            sum_ps = ps_c.tile([128, 32], F32, tag="sum")
            nc.tensor.matmul(sum_ps[:], s_t[:], msg_sb[:], start=True, stop=True)

            mean_sb = wpool.tile([128, 32], F32, tag="mean")
            nc.scalar.activation(
                mean_sb[:],
                sum_ps[:],
                mybir.ActivationFunctionType.Copy,
                bias=0.0,
                scale=rrt[:, 1:2],
            )
            pb_sb = wpool.tile([128, 32], F32, tag="pb")
            nc.gpsimd.tensor_add(pb_sb[:], mean_sb[:], bb_t[:])
            ot = opool.tile([128, 32], F32, tag="o")
            nc.scalar.activation(ot[:], pb_sb[:], mybir.ActivationFunctionType.Relu)
            nc.sync.dma_start(out_d[t], ot[:])

    nc.compile()
    return nc


_BUILD_CACHE = {}


def _built(T):
    nc = _BUILD_CACHE.get(T)
    if nc is None:
        nc = _build(T)
        _BUILD_CACHE[T] = nc
    return nc


def _finalize(results, meta):
    rows = np.concatenate([r["out"] for r in results], axis=0)  # [NCORES*T,EPT,D]
    rows = rows.reshape(-1, D)
    b = meta["bias"]
    out = np.empty((meta["n_nodes"], D), np.float32)
    out[:] = np.maximum(b, 0.0)[None, :]
    out[meta["nodes_k"]] = rows[meta["flatslot"]]
    if meta["host_rows"] is not None:
        out[meta["host_nodes"]] = meta["host_rows"]
    return out


def kernel(node_states, edge_index, a_in, bias):
    in_maps, meta = _prep(node_states, edge_index, a_in, bias)
    nc = _built(meta["T"])
    res = run_bass_kernel_spmd(nc, in_maps, list(range(NCORES)))
    return _finalize(res.results, meta)


if __name__ == "__main__":
    # smoke test on random small data via CoreSim unless RUN_HW=1
    np.random.seed(0)
    n_nodes, n_edges = 700, 3000
    ns = np.random.randn(n_nodes, D).astype(np.float32)
    ei = np.random.randint(0, n_nodes, (n_edges, 2)).astype(np.int64)
    a = (np.random.randn(n_edges, D, D) / np.sqrt(D)).astype(np.float32)
    b = np.random.uniform(-0.2, 0.2, D).astype(np.float32)

    x_i = ns[ei[:, 0]]
    msg = np.einsum("ed,edf->ef", x_i, a)
    summed = np.zeros((n_nodes, D), np.float32)
    np.add.at(summed, ei[:, 1], msg)
    cnt = np.bincount(ei[:, 1], minlength=n_nodes).astype(np.float32)
    expected = np.maximum(summed / np.maximum(cnt, 1.0)[:, None] + b[None, :], 0.0)

    if os.environ.get("RUN_HW"):
        actual = kernel(ns, ei, a, b)
    else:
        from concourse.bass_interp import CoreSim

        in_maps, meta = _prep(ns, ei, a, b)
        nc = _build(meta["T"], enable_asserts=True)
        outs = []
        for c in range(NCORES):
            sim = CoreSim(nc, trace=False)
            for k, v in in_maps[c].items():
                sim.tensor(k)[:] = v
            sim.simulate()
            outs.append({"out": np.array(sim.tensor("out"))})
        actual = _finalize(outs, meta)

    err = np.abs(actual - expected)
    denom = np.abs(expected).max()
    print("max abs err:", err.max(), "rel to scale:", err.max() / denom)
    rel = np.linalg.norm(actual - expected) / np.linalg.norm(expected)
    print("l2 rel:", rel)
    assert err.max() / denom < 1e-4, "FAIL"
    print("PASS")
